# revision 28
# baseline (speedup 1.0000x reference)
"""Trainium2 Bass kernel for a causal attention head block (B=4, T=2048, C=2048,
H=16, D=128) with RoPE (single fixed position, folded into weights on host).

Sharding: 8 cores = 4 batches x 2 head-groups (8 heads each).
Per core: QKV projection (f32r matmuls), causal attention with exp-softmax
(no max subtraction -- scores are small), out-projection partial. Host sums
the two per-batch partials and adds the folded bias.

All matmuls run in float32r (TRN2 reduced-precision fp32 mode): ~bf16 speed at
moving-dim >= 256, ~1.5e-4 relative accuracy per matmul.
"""
import numpy as np

B, T, C, H, D = 4, 2048, 2048, 16, 128
ROPE_BASE = 10000.0
HG = H // 2            # heads per core: 8
JQ = HG * D            # 1024 q (or k, or v) channels per core
NCORES = 8
NCT = C // 128         # 16 contraction tiles
NTT = T // 128         # 16 token tiles
NTC = T // 512         # 4 token chunks of 512

_CACHE = {}


def _build_nc():
    import concourse.bass as bass
    import concourse.mybir as mybir
    import concourse.tile as tile
    from concourse import bacc

    f32, f32r = mybir.dt.float32, mybir.dt.float32r
    ds, ts = bass.ds, bass.ts
    Exp = mybir.ActivationFunctionType.Exp
    Ident = mybir.ActivationFunctionType.Identity
    mult = mybir.AluOpType.mult
    add = mybir.AluOpType.add

    nc = bacc.Bacc("TRN2", target_bir_lowering=False, debug=False)
    xT = nc.dram_tensor("xT", [C, T], f32r, kind="ExternalInput").ap()
    WqkT = nc.dram_tensor("WqkT", [C, 2 * JQ], f32r, kind="ExternalInput").ap()
    WvT = nc.dram_tensor("WvT", [C, JQ], f32r, kind="ExternalInput").ap()
    WoT = nc.dram_tensor("WoT", [JQ, C], f32r, kind="ExternalInput").ap()
    bq = nc.dram_tensor("bq", [JQ, 1], f32, kind="ExternalInput").ap()
    msk = nc.dram_tensor("msk", [128, 4 * 512], f32r, kind="ExternalInput").ap()
    ones_in = nc.dram_tensor("ones_in", [128, 128], f32r, kind="ExternalInput").ap()
    qk_sp = nc.dram_tensor("qk_sp", [2 * JQ, T], f32r).ap()   # [q;k]^T spill
    v_sp = nc.dram_tensor("v_sp", [T, JQ], f32r).ap()         # V spill [t, jv]
    o = nc.dram_tensor("o", [T, C], f32, kind="ExternalOutput").ap()

    with tile.TileContext(nc) as tc:
        with tc.tile_pool(name="const", bufs=1) as cpool:
            mask_t = cpool.tile([128, 4 * 512], f32r, tag="mask")
            nc.sync.dma_start(mask_t[:], msk[:])
            ones2d = cpool.tile([128, 128], f32r, tag="ones2d")
            nc.sync.dma_start(ones2d[:], ones_in[:])
            ones_col = ones2d[:, 0:1]
            ones_row = ones2d[0:1, :]
            bq_t = []
            for j in range(JQ // 128):
                t_ = cpool.tile([128, 1], f32, tag=f"bq{j}")
                nc.sync.dma_start(t_[:], bq[ts(j, 128), :])
                bq_t.append(t_)

            # PE warmup: dummy fp32 matmuls with no input deps keep the
            # HAM activity monitor busy during the initial DMA wait so
            # phase A's first real matmuls run at full clock.
            wtile = cpool.tile([128, 640], f32, tag="warm")
            nc.vector.memset(wtile[:], 0.0)
            with tc.tile_pool(name="pwarm", bufs=1, space="PSUM") as pwarm:
                ps_w = pwarm.tile([128, 512], f32, tag="pw", name="pw")
                for wi in range(8):
                    nc.tensor.matmul(ps_w[:], wtile[:, 0:128],
                                     wtile[:, 128:640], start=(wi == 0),
                                     stop=(wi == 7))

            # ---------------- Phase A: projections ----------------
            # Head-0 attention inputs live below the xt stack slot so their
            # DMAs can prefetch during phase A (disjoint SBUF space).
            kv0cm = tc.tile_pool(name="kv0", bufs=1)
            kv0pool = kv0cm.__enter__()
            kt0 = kv0pool.tile([128, T], f32r, tag="kt0", name="kt0")
            vh0 = kv0pool.tile([128, T], f32r, tag="vh0", name="vh0")
            qc0 = kv0pool.tile([128, 512], f32r, tag="qc0", name="qc0")
            with tc.tile_pool(name="xt", bufs=1) as xpool:
                # A-V first: V[t, jv] = x^T[c, t].T @ Wv^T[c, jv].
                # Interleave xT and first-chunk WvT DMAs so PE starts early.
                xt = [None] * NCT
                with tc.tile_pool(name="wv", bufs=19) as wvpool, \
                     tc.tile_pool(name="vst", bufs=4) as vspool, \
                     tc.tile_pool(name="psV", bufs=5, space="PSUM") as psvpool:
                    for vch in range(JQ // 512):     # 2 chunks of 512
                        wvs = []
                        for ci in range(NCT):
                            if vch == 0:
                                t_ = xpool.tile([128, T], f32r, tag=f"x{ci}",
                                                name=f"x{ci}")
                                nc.sync.dma_start(t_[:], xT[ts(ci, 128), :])
                                xt[ci] = t_
                            w_ = wvpool.tile([128, 512], f32r, tag="wv",
                                             name=f"wv{vch}_{ci}")
                            nc.sync.dma_start(
                                w_[:], WvT[ts(ci, 128), ds(vch * 512, 512)])
                            wvs.append(w_)
                        for tt in range(NTT):
                            ps = psvpool.tile([128, 512], f32, tag="psv")
                            for ci in range(NCT):
                                nc.tensor.matmul(
                                    ps[:], xt[ci][:, ts(tt, 128)], wvs[ci][:],
                                    start=(ci == 0), stop=(ci == NCT - 1))
                            st = vspool.tile([128, 512], f32r, tag="vst")
                            nc.vector.tensor_copy(st[:], ps[:])
                            nc.gpsimd.dma_start(
                                v_sp[ts(tt, 128), ds(vch * 512, 512)], st[:])
                # A-QK: qk^T[j, t] = Wqk^T[c, j].T @ x^T[c, t]  (+ bias on q)
                # Group order q0, k0, q1, k1 so attention heads 0-3 unblock
                # after two groups.
                with tc.tile_pool(name="wqk", bufs=18) as wpool, \
                     tc.tile_pool(name="qkst", bufs=6) as spool, \
                     tc.tile_pool(name="psA", bufs=5, space="PSUM") as pspool:
                    for jg_i, jg in enumerate((0, 2, 1, 3)):
                        wts = []
                        for ci in range(NCT):
                            w_ = wpool.tile([128, 512], f32r, tag="w",
                                            name=f"w{jg}_{ci}")
                            nc.sync.dma_start(
                                w_[:], WqkT[ts(ci, 128), ds(jg * 512, 512)])
                            wts.append(w_)
                        for jj in range(4):
                            jt = jg * 4 + jj
                            for tch in range(NTC):
                                ps = pspool.tile([128, 512], f32, tag="ps")
                                for ci in range(NCT):
                                    nc.tensor.matmul(
                                        ps[:], wts[ci][:, ts(jj, 128)],
                                        xt[ci][:, ts(tch, 512)],
                                        start=(ci == 0), stop=(ci == NCT - 1))
                                st = spool.tile([128, 512], f32r, tag="st")
                                if jt < JQ // 128:   # q tile: bias add
                                    nc.scalar.activation(
                                        st[:], ps[:], Ident,
                                        bias=bq_t[jt][:, 0:1])
                                else:                # k tile: plain copy
                                    nc.scalar.copy(st[:], ps[:])
                                nc.gpsimd.dma_start(
                                    qk_sp[ts(jt, 128), ts(tch, 512)], st[:])
                        if jg_i == 1:
                            # prefetch head-0 attention inputs mid-phase-A
                            nc.sync.dma_start(
                                kt0[:], qk_sp[ds(JQ, 128), :])
                            nc.sync.dma_start(
                                vh0[:].rearrange("p (n d) -> p n d", d=128),
                                v_sp[:, ds(0, 128)].rearrange(
                                    "(n p) d -> p n d", p=128))
                            nc.sync.dma_start(qc0[:], qk_sp[ds(0, 128),
                                                            ds(0, 512)])

            # ---------------- Phase B: attention ----------------
            with tc.tile_pool(name="ysb", bufs=1) as ypool:
                y_t = [ypool.tile([128, T], f32r, tag=f"y{h}", name=f"y{h}")
                       for h in range(HG)]
                with tc.tile_pool(name="kv", bufs=2) as kvpool, \
                     tc.tile_pool(name="qc", bufs=2) as qcpool, \
                     tc.tile_pool(name="es", bufs=6) as espool, \
                     tc.tile_pool(name="nrm", bufs=3) as npool, \
                     tc.tile_pool(name="psS", bufs=3, space="PSUM") as pss, \
                     tc.tile_pool(name="psY", bufs=2, space="PSUM") as psy, \
                     tc.tile_pool(name="psD", bufs=3, space="PSUM") as psd:

                    def emit_norm(pend):
                        # Deferred per-chunk softmax normalization: runs the
                        # reciprocal off the PE critical path (the broadcast
                        # matmul is emitted a few matmuls into the NEXT chunk,
                        # by which time the DVE reciprocal has completed).
                        ps_y_, ps_d_, h_, ci_ = pend
                        rec = npool.tile([1, 512], f32, tag="rec", name="rec")
                        nc.vector.reciprocal_approx_fast(rec[:], ps_d_[:])
                        recr = npool.tile([1, 512], f32r, tag="recr",
                                          name="recr")
                        nc.vector.tensor_copy(recr[:], rec[:])
                        ps_b = pss.tile([128, 512], f32, tag="pss", name="pb")
                        nc.tensor.matmul(ps_b[:], ones_row, recr[:],
                                         start=True, stop=True)
                        bc = npool.tile([128, 512], f32r, tag="bc", name="bc")
                        nc.scalar.copy(bc[:], ps_b[:])
                        nc.vector.tensor_tensor(
                            y_t[h_][:, ds(ci_ * 512, 512)], ps_y_[:], bc[:],
                            mult)

                    pending = None
                    carry = None
                    for h in range(HG):
                        if h == 0:
                            kt = kt0
                        else:
                            kt = kvpool.tile([128, T], f32r, tag="kt")
                            nc.sync.dma_start(
                                kt[:], qk_sp[ds(JQ + h * 128, 128), :])
                        if h == 0:
                            vh = vh0
                        else:
                            vh = kvpool.tile([128, T], f32r, tag="vh")
                            nc.sync.dma_start(
                                vh[:].rearrange("p (n d) -> p n d", d=128),
                                v_sp[:, ds(h * 128, 128)].rearrange(
                                    "(n p) d -> p n d", p=128))
                        for ci in range(NTC):
                            if h == 0 and ci == 0:
                                qc = qc0
                            else:
                                qc = qcpool.tile([128, 512], f32r, tag="qc")
                                nc.sync.dma_start(
                                    qc[:],
                                    qk_sp[ds(h * 128, 128), ds(ci * 512, 512)])
                            ps_y = psy.tile([128, 512], f32, tag="py")
                            ps_d = psd.tile([1, 512], f32, tag="pd")
                            njt = 4 * (ci + 1)
                            norm_at = min(4, njt - 1)
                            for jt in range(njt):
                                jx = jt
                                ps_s = pss.tile([128, 512], f32, tag="pss")
                                nc.tensor.matmul(ps_s[:], kt[:, ts(jt, 128)],
                                                 qc[:], start=True, stop=True)
                                if jx == norm_at and pending is not None:
                                    emit_norm(pending)
                                    pending = None
                                if carry is not None:
                                    (c_es, c_py, c_pd, c_vs, c_st, c_first,
                                     c_h, c_ci) = carry
                                    nc.tensor.matmul(
                                        c_pd[:], ones_col, c_es[:],
                                        start=c_first, stop=c_st)
                                    nc.tensor.matmul(
                                        c_py[:], c_vs, c_es[:],
                                        start=c_first, stop=c_st)
                                    if c_st:
                                        pending = (c_py, c_pd, c_h, c_ci)
                                es = espool.tile([128, 512], f32r, tag="es")
                                nc.scalar.activation(es[:], ps_s[:], Exp)
                                off = jt * 128 - ci * 512
                                if off >= 0:   # diagonal band: apply mask
                                    oi = off // 128
                                    nc.vector.tensor_tensor(
                                        es[:], es[:],
                                        mask_t[:, ds(oi * 512, 512)], mult)
                                carry = (es, ps_y, ps_d, vh[:, ts(jt, 128)],
                                         jx == njt - 1, jx == 0, h, ci)
                    # drain the last carry + norm
                    c_es, c_py, c_pd, c_vs, c_st, c_first, c_h, c_ci = carry
                    nc.tensor.matmul(c_pd[:], ones_col, c_es[:],
                                     start=c_first, stop=True)
                    nc.tensor.matmul(c_py[:], c_vs, c_es[:],
                                     start=c_first, stop=True)
                    if pending is not None:
                        emit_norm(pending)
                    emit_norm((c_py, c_pd, c_h, c_ci))

                # ---------------- Phase C: out-projection ----------------
                with tc.tile_pool(name="woc", bufs=18) as wopool, \
                     tc.tile_pool(name="ost", bufs=4) as ospool, \
                     tc.tile_pool(name="psO", bufs=3, space="PSUM") as pso:
                    for cch in range(C // 512):
                        wo_t = []
                        for ch in range(HG):
                            w_ = wopool.tile([128, 512], f32r, tag="woc",
                                             name=f"woc{cch}_{ch}")
                            nc.sync.dma_start(
                                w_[:], WoT[ts(ch, 128), ds(cch * 512, 512)])
                            wo_t.append(w_)
                        for tt in range(NTT):
                            ps = pso.tile([128, 512], f32, tag="po")
                            for ch in range(HG):
                                nc.tensor.matmul(
                                    ps[:], y_t[ch][:, ts(tt, 128)],
                                    wo_t[ch][:], start=(ch == 0),
                                    stop=(ch == HG - 1))
                            st = ospool.tile([128, 512], f32, tag="ost")
                            nc.vector.tensor_copy(st[:], ps[:])
                            nc.gpsimd.dma_start(
                                o[ts(tt, 128), ds(cch * 512, 512)], st[:])
            kv0cm.__exit__(None, None, None)
    nc.compile()
    return nc


def _rope_matrix():
    inv_freq = 1.0 / (ROPE_BASE ** (np.arange(0, D, 2, dtype=np.float64) / D))
    freqs = np.float64(T) * inv_freq
    emb = np.concatenate([freqs, freqs])
    cos, sin = np.cos(emb), np.sin(emb)
    R = np.zeros((D, D))
    for j in range(D):
        R[j, j] = cos[j]
    for j in range(64):
        R[j, 2 * j + 1] += -sin[j]
    for j in range(64, 128):
        R[j, 2 * (j - 64)] += sin[j]
    return R


def _host_mask():
    m = np.zeros((128, 4 * 512), dtype=np.float32)
    ii = np.arange(512)
    jj = np.arange(128)
    for oi, off in enumerate((0, 128, 256, 384)):
        m[:, oi * 512:(oi + 1) * 512] = (jj[:, None] + off <= ii[None, :])
    return m


def kernel(x, Wqkv, bqkv, Wout, bout):
    from concourse.bass_utils import run_bass_kernel_spmd

    if "nc" not in _CACHE:
        _CACHE["nc"] = _build_nc()
    nc = _CACHE["nc"]

    x = np.asarray(x, dtype=np.float32)
    Wqkv64 = np.asarray(Wqkv, dtype=np.float64)
    bqkv64 = np.asarray(bqkv, dtype=np.float64)
    Wout64 = np.asarray(Wout, dtype=np.float64)
    bout64 = np.asarray(bout, dtype=np.float64)

    R = _rope_matrix()
    scale = 1.0 / np.sqrt(np.float64(D))
    Wq = Wqkv64[:C].reshape(H, D, C)
    Wk = Wqkv64[C:2 * C].reshape(H, D, C)
    Wv = Wqkv64[2 * C:].reshape(H, D, C)
    bqv = bqkv64[:C].reshape(H, D)
    bv = bqkv64[2 * C:]

    Wq_f = np.einsum('jk,hkc->hjc', R, Wq) * scale
    bq_f = np.einsum('jk,hk->hj', R, bqv) * scale
    Wk_f = np.einsum('jk,hkc->hjc', R, Wk)
    bias_final = (bout64 + Wout64 @ bv).astype(np.float32)

    mask = _host_mask()
    in_maps = []
    xTb = [np.ascontiguousarray(x[b].T) for b in range(B)]
    shard = {}
    for hg in range(2):
        hs = slice(hg * HG, (hg + 1) * HG)
        wqk = np.concatenate(
            [Wq_f[hs].reshape(JQ, C), Wk_f[hs].reshape(JQ, C)], axis=0)
        shard[hg] = dict(
            WqkT=np.ascontiguousarray(wqk.T.astype(np.float32)),
            WvT=np.ascontiguousarray(
                Wv[hs].reshape(JQ, C).T.astype(np.float32)),
            WoT=np.ascontiguousarray(
                Wout64[:, hg * JQ:(hg + 1) * JQ].T.astype(np.float32)),
            bq=bq_f[hs].reshape(JQ, 1).astype(np.float32),
        )
    for core in range(NCORES):
        b, hg = core // 2, core % 2
        in_maps.append(dict(xT=xTb[b], msk=mask,
                            ones_in=np.ones((128, 128), dtype=np.float32),
                            **shard[hg]))

    res = run_bass_kernel_spmd(nc, in_maps, core_ids=list(range(NCORES)),
                               **_CACHE.get("run_kwargs", {}))
    _CACHE["last_result"] = res
    out = np.empty((B, T, C), dtype=np.float32)
    for b in range(B):
        out[b] = (res.results[2 * b]["o"] + res.results[2 * b + 1]["o"]
                  + bias_final)
    return out


# revision 29
# speedup vs baseline: 1.1517x; 1.1517x over previous
"""Trainium2 Bass kernel for a causal attention head block (B=4, T=2048, C=2048,
H=16, D=128) with RoPE (single fixed position, folded into weights on host).

Sharding: 8 cores = 4 batches x 2 head-groups (8 heads each).
Per core: QKV projection (f32r matmuls), causal attention with exp-softmax
(no max subtraction -- scores are small), out-projection partial. Host sums
the two per-batch partials and adds the folded bias.

All matmuls run in float32r (TRN2 reduced-precision fp32 mode): ~bf16 speed at
moving-dim >= 256, ~1.5e-4 relative accuracy per matmul.
"""
import numpy as np

B, T, C, H, D = 4, 2048, 2048, 16, 128
ROPE_BASE = 10000.0
HG = H // 2            # heads per core: 8
JQ = HG * D            # 1024 q (or k, or v) channels per core
NCORES = 8
NCT = C // 128         # 16 contraction tiles
NTT = T // 128         # 16 token tiles
NTC = T // 512         # 4 token chunks of 512

_CACHE = {}


def _build_nc():
    import concourse.bass as bass
    import concourse.mybir as mybir
    import concourse.tile as tile
    from concourse import bacc

    f32, f32r = mybir.dt.float32, mybir.dt.float32r
    ds, ts = bass.ds, bass.ts
    Exp = mybir.ActivationFunctionType.Exp
    Ident = mybir.ActivationFunctionType.Identity
    mult = mybir.AluOpType.mult
    add = mybir.AluOpType.add

    nc = bacc.Bacc("TRN2", target_bir_lowering=False, debug=False)
    xT = nc.dram_tensor("xT", [C, T], f32r, kind="ExternalInput").ap()
    WqkT = nc.dram_tensor("WqkT", [C, 2 * JQ], f32r, kind="ExternalInput").ap()
    WvT = nc.dram_tensor("WvT", [C, JQ], f32r, kind="ExternalInput").ap()
    WoT = nc.dram_tensor("WoT", [JQ, C], f32r, kind="ExternalInput").ap()
    bq = nc.dram_tensor("bq", [JQ, 1], f32, kind="ExternalInput").ap()
    msk = nc.dram_tensor("msk", [128, 4 * 512], f32r, kind="ExternalInput").ap()
    ones_in = nc.dram_tensor("ones_in", [128, 128], f32r, kind="ExternalInput").ap()
    qk_sp = nc.dram_tensor("qk_sp", [2 * JQ, T], f32r).ap()   # [q;k]^T spill
    v_sp = nc.dram_tensor("v_sp", [T, JQ], f32r).ap()         # V spill [t, jv]
    o = nc.dram_tensor("o", [T, C], f32, kind="ExternalOutput").ap()

    with tile.TileContext(nc) as tc:
        with tc.tile_pool(name="const", bufs=1) as cpool:
            mask_t = cpool.tile([128, 4 * 512], f32r, tag="mask")
            nc.sync.dma_start(mask_t[:], msk[:])
            ones2d = cpool.tile([128, 128], f32r, tag="ones2d")
            nc.sync.dma_start(ones2d[:], ones_in[:])
            ones_col = ones2d[:, 0:1]
            ones_row = ones2d[0:1, :]
            bq_t = []
            for j in range(JQ // 128):
                t_ = cpool.tile([128, 1], f32, tag=f"bq{j}")
                nc.sync.dma_start(t_[:], bq[ts(j, 128), :])
                bq_t.append(t_)

            # PE warmup: dummy fp32 matmuls with no input deps keep the
            # HAM activity monitor busy during the initial DMA wait so
            # phase A's first real matmuls run at full clock.
            wtile = cpool.tile([128, 640], f32, tag="warm")
            nc.vector.memset(wtile[:], 0.0)
            with tc.tile_pool(name="pwarm", bufs=1, space="PSUM") as pwarm:
                ps_w = pwarm.tile([128, 512], f32, tag="pw", name="pw")
                for wi in range(8):
                    nc.tensor.matmul(ps_w[:], wtile[:, 0:128],
                                     wtile[:, 128:640], start=(wi == 0),
                                     stop=(wi == 7))

            # ---------------- Phase A: projections ----------------
            # Head-0 attention inputs live below the xt stack slot so their
            # DMAs can prefetch during phase A (disjoint SBUF space).
            kv0cm = tc.tile_pool(name="kv0", bufs=1)
            kv0pool = kv0cm.__enter__()
            kt0 = kv0pool.tile([128, T], f32r, tag="kt0", name="kt0")
            vh0 = kv0pool.tile([128, T], f32r, tag="vh0", name="vh0")
            qc0 = kv0pool.tile([128, 512], f32r, tag="qc0", name="qc0")
            with tc.tile_pool(name="xt", bufs=1) as xpool:
                # A-V first: V[t, jv] = x^T[c, t].T @ Wv^T[c, jv].
                # Interleave xT and first-chunk WvT DMAs so PE starts early.
                xt = [None] * NCT
                with tc.tile_pool(name="wv", bufs=19) as wvpool, \
                     tc.tile_pool(name="vst", bufs=4) as vspool, \
                     tc.tile_pool(name="psV", bufs=5, space="PSUM") as psvpool:
                    for vch in range(JQ // 512):     # 2 chunks of 512
                        wvs = []
                        for ci in range(NCT):
                            if vch == 0:
                                t_ = xpool.tile([128, T], f32r, tag=f"x{ci}",
                                                name=f"x{ci}")
                                nc.sync.dma_start(t_[:], xT[ts(ci, 128), :])
                                xt[ci] = t_
                            w_ = wvpool.tile([128, 512], f32r, tag="wv",
                                             name=f"wv{vch}_{ci}")
                            nc.sync.dma_start(
                                w_[:], WvT[ts(ci, 128), ds(vch * 512, 512)])
                            wvs.append(w_)
                        for tt in range(NTT):
                            ps = psvpool.tile([128, 512], f32, tag="psv")
                            for ci in range(NCT):
                                nc.tensor.matmul(
                                    ps[:], xt[ci][:, ts(tt, 128)], wvs[ci][:],
                                    start=(ci == 0), stop=(ci == NCT - 1))
                            st = vspool.tile([128, 512], f32r, tag="vst")
                            nc.vector.tensor_copy(st[:], ps[:])
                            nc.gpsimd.dma_start(
                                v_sp[ts(tt, 128), ds(vch * 512, 512)], st[:])
                # A-QK: qk^T[j, t] = Wqk^T[c, j].T @ x^T[c, t]  (+ bias on q)
                # Group order q0, k0, q1, k1 so attention heads 0-3 unblock
                # after two groups.
                with tc.tile_pool(name="wqk", bufs=18) as wpool, \
                     tc.tile_pool(name="qkst", bufs=6) as spool, \
                     tc.tile_pool(name="psA", bufs=5, space="PSUM") as pspool:
                    for jg_i, jg in enumerate((0, 2, 1, 3)):
                        wts = []
                        for ci in range(NCT):
                            w_ = wpool.tile([128, 512], f32r, tag="w",
                                            name=f"w{jg}_{ci}")
                            nc.sync.dma_start(
                                w_[:], WqkT[ts(ci, 128), ds(jg * 512, 512)])
                            wts.append(w_)
                        for jj in range(4):
                            jt = jg * 4 + jj
                            for tch in range(NTC):
                                ps = pspool.tile([128, 512], f32, tag="ps")
                                for ci in range(NCT):
                                    nc.tensor.matmul(
                                        ps[:], wts[ci][:, ts(jj, 128)],
                                        xt[ci][:, ts(tch, 512)],
                                        start=(ci == 0), stop=(ci == NCT - 1))
                                st = spool.tile([128, 512], f32r, tag="st")
                                if jt < JQ // 128:   # q tile: bias add
                                    nc.scalar.activation(
                                        st[:], ps[:], Ident,
                                        bias=bq_t[jt][:, 0:1])
                                else:                # k tile: plain copy
                                    nc.scalar.copy(st[:], ps[:])
                                nc.gpsimd.dma_start(
                                    qk_sp[ts(jt, 128), ts(tch, 512)], st[:])
                        if jg_i == 1:
                            # prefetch head-0 attention inputs mid-phase-A
                            nc.sync.dma_start(
                                kt0[:], qk_sp[ds(JQ, 128), :])
                            nc.sync.dma_start(
                                vh0[:].rearrange("p (n d) -> p n d", d=128),
                                v_sp[:, ds(0, 128)].rearrange(
                                    "(n p) d -> p n d", p=128))
                            nc.sync.dma_start(qc0[:], qk_sp[ds(0, 128),
                                                            ds(0, 512)])

            # ---------------- Phase B: attention ----------------
            with tc.tile_pool(name="ysb", bufs=1) as ypool:
                y_t = [ypool.tile([128, T], f32r, tag=f"y{h}", name=f"y{h}")
                       for h in range(HG)]
                with tc.tile_pool(name="kv", bufs=2) as kvpool, \
                     tc.tile_pool(name="qc", bufs=2) as qcpool, \
                     tc.tile_pool(name="es", bufs=6) as espool, \
                     tc.tile_pool(name="nrm", bufs=3) as npool, \
                     tc.tile_pool(name="psS", bufs=3, space="PSUM") as pss, \
                     tc.tile_pool(name="psY", bufs=2, space="PSUM") as psy, \
                     tc.tile_pool(name="psD", bufs=2, space="PSUM") as psd, \
                     tc.tile_pool(name="psB", bufs=1, space="PSUM") as psb:

                    def emit_norm(pend):
                        # Deferred per-chunk softmax normalization: runs the
                        # reciprocal off the PE critical path (the broadcast
                        # matmul is emitted a few matmuls into the NEXT chunk,
                        # by which time the DVE reciprocal has completed).
                        ps_y_, ps_d_, h_, ci_ = pend
                        rec = npool.tile([1, 512], f32, tag="rec", name="rec")
                        nc.vector.reciprocal_approx_fast(rec[:], ps_d_[:])
                        recr = npool.tile([1, 512], f32r, tag="recr",
                                          name="recr")
                        nc.vector.tensor_copy(recr[:], rec[:])
                        ps_b = psb.tile([128, 512], f32, tag="pb", name="pb")
                        nc.tensor.matmul(ps_b[:], ones_row, recr[:],
                                         start=True, stop=True)
                        bc = npool.tile([128, 512], f32r, tag="bc", name="bc")
                        nc.scalar.copy(bc[:], ps_b[:])
                        nc.vector.tensor_tensor(
                            y_t[h_][:, ds(ci_ * 512, 512)], ps_y_[:], bc[:],
                            mult)

                    pending = None
                    carry = None
                    for h in range(HG):
                        if h == 0:
                            kt = kt0
                        else:
                            kt = kvpool.tile([128, T], f32r, tag="kt")
                            nc.sync.dma_start(
                                kt[:], qk_sp[ds(JQ + h * 128, 128), :])
                        if h == 0:
                            vh = vh0
                        else:
                            vh = kvpool.tile([128, T], f32r, tag="vh")
                            nc.sync.dma_start(
                                vh[:].rearrange("p (n d) -> p n d", d=128),
                                v_sp[:, ds(h * 128, 128)].rearrange(
                                    "(n p) d -> p n d", p=128))
                        for ci in range(NTC):
                            if h == 0 and ci == 0:
                                qc = qc0
                            else:
                                qc = qcpool.tile([128, 512], f32r, tag="qc")
                                nc.sync.dma_start(
                                    qc[:],
                                    qk_sp[ds(h * 128, 128), ds(ci * 512, 512)])
                            ps_y = psy.tile([128, 512], f32, tag="py")
                            ps_d = psd.tile([1, 512], f32, tag="pd")
                            njt = 4 * (ci + 1)
                            norm_at = min(4, njt - 1)
                            for jt in range(njt):
                                jx = jt
                                ps_s = pss.tile([128, 512], f32, tag="pss")
                                nc.tensor.matmul(ps_s[:], kt[:, ts(jt, 128)],
                                                 qc[:], start=True, stop=True)
                                if jx == norm_at and pending is not None:
                                    emit_norm(pending)
                                    pending = None
                                if carry is not None:
                                    (c_es, c_py, c_pd, c_vs, c_st, c_first,
                                     c_h, c_ci) = carry
                                    nc.tensor.matmul(
                                        c_pd[:], ones_col, c_es[:],
                                        start=c_first, stop=c_st)
                                    nc.tensor.matmul(
                                        c_py[:], c_vs, c_es[:],
                                        start=c_first, stop=c_st)
                                    if c_st:
                                        pending = (c_py, c_pd, c_h, c_ci)
                                es = espool.tile([128, 512], f32r, tag="es")
                                nc.scalar.activation(es[:], ps_s[:], Exp)
                                off = jt * 128 - ci * 512
                                if off >= 0:   # diagonal band: apply mask
                                    oi = off // 128
                                    nc.vector.tensor_tensor(
                                        es[:], es[:],
                                        mask_t[:, ds(oi * 512, 512)], mult)
                                carry = (es, ps_y, ps_d, vh[:, ts(jt, 128)],
                                         jx == njt - 1, jx == 0, h, ci)
                    # drain the last carry + norm
                    c_es, c_py, c_pd, c_vs, c_st, c_first, c_h, c_ci = carry
                    nc.tensor.matmul(c_pd[:], ones_col, c_es[:],
                                     start=c_first, stop=True)
                    nc.tensor.matmul(c_py[:], c_vs, c_es[:],
                                     start=c_first, stop=True)
                    if pending is not None:
                        emit_norm(pending)
                    emit_norm((c_py, c_pd, c_h, c_ci))

                # ---------------- Phase C: out-projection ----------------
                with tc.tile_pool(name="woc", bufs=18) as wopool, \
                     tc.tile_pool(name="ost", bufs=4) as ospool, \
                     tc.tile_pool(name="psO", bufs=3, space="PSUM") as pso:
                    for cch in range(C // 512):
                        wo_t = []
                        for ch in range(HG):
                            w_ = wopool.tile([128, 512], f32r, tag="woc",
                                             name=f"woc{cch}_{ch}")
                            nc.sync.dma_start(
                                w_[:], WoT[ts(ch, 128), ds(cch * 512, 512)])
                            wo_t.append(w_)
                        for tt in range(NTT):
                            ps = pso.tile([128, 512], f32, tag="po")
                            for ch in range(HG):
                                nc.tensor.matmul(
                                    ps[:], y_t[ch][:, ts(tt, 128)],
                                    wo_t[ch][:], start=(ch == 0),
                                    stop=(ch == HG - 1))
                            st = ospool.tile([128, 512], f32, tag="ost")
                            nc.vector.tensor_copy(st[:], ps[:])
                            nc.gpsimd.dma_start(
                                o[ts(tt, 128), ds(cch * 512, 512)], st[:])
            kv0cm.__exit__(None, None, None)
    nc.compile()
    return nc


def _rope_matrix():
    inv_freq = 1.0 / (ROPE_BASE ** (np.arange(0, D, 2, dtype=np.float64) / D))
    freqs = np.float64(T) * inv_freq
    emb = np.concatenate([freqs, freqs])
    cos, sin = np.cos(emb), np.sin(emb)
    R = np.zeros((D, D))
    for j in range(D):
        R[j, j] = cos[j]
    for j in range(64):
        R[j, 2 * j + 1] += -sin[j]
    for j in range(64, 128):
        R[j, 2 * (j - 64)] += sin[j]
    return R


def _host_mask():
    m = np.zeros((128, 4 * 512), dtype=np.float32)
    ii = np.arange(512)
    jj = np.arange(128)
    for oi, off in enumerate((0, 128, 256, 384)):
        m[:, oi * 512:(oi + 1) * 512] = (jj[:, None] + off <= ii[None, :])
    return m


def kernel(x, Wqkv, bqkv, Wout, bout):
    from concourse.bass_utils import run_bass_kernel_spmd

    if "nc" not in _CACHE:
        _CACHE["nc"] = _build_nc()
    nc = _CACHE["nc"]

    x = np.asarray(x, dtype=np.float32)
    Wqkv64 = np.asarray(Wqkv, dtype=np.float64)
    bqkv64 = np.asarray(bqkv, dtype=np.float64)
    Wout64 = np.asarray(Wout, dtype=np.float64)
    bout64 = np.asarray(bout, dtype=np.float64)

    R = _rope_matrix()
    scale = 1.0 / np.sqrt(np.float64(D))
    Wq = Wqkv64[:C].reshape(H, D, C)
    Wk = Wqkv64[C:2 * C].reshape(H, D, C)
    Wv = Wqkv64[2 * C:].reshape(H, D, C)
    bqv = bqkv64[:C].reshape(H, D)
    bv = bqkv64[2 * C:]

    Wq_f = np.einsum('jk,hkc->hjc', R, Wq) * scale
    bq_f = np.einsum('jk,hk->hj', R, bqv) * scale
    Wk_f = np.einsum('jk,hkc->hjc', R, Wk)
    bias_final = (bout64 + Wout64 @ bv).astype(np.float32)

    mask = _host_mask()
    in_maps = []
    xTb = [np.ascontiguousarray(x[b].T) for b in range(B)]
    shard = {}
    for hg in range(2):
        hs = slice(hg * HG, (hg + 1) * HG)
        wqk = np.concatenate(
            [Wq_f[hs].reshape(JQ, C), Wk_f[hs].reshape(JQ, C)], axis=0)
        shard[hg] = dict(
            WqkT=np.ascontiguousarray(wqk.T.astype(np.float32)),
            WvT=np.ascontiguousarray(
                Wv[hs].reshape(JQ, C).T.astype(np.float32)),
            WoT=np.ascontiguousarray(
                Wout64[:, hg * JQ:(hg + 1) * JQ].T.astype(np.float32)),
            bq=bq_f[hs].reshape(JQ, 1).astype(np.float32),
        )
    for core in range(NCORES):
        b, hg = core // 2, core % 2
        in_maps.append(dict(xT=xTb[b], msk=mask,
                            ones_in=np.ones((128, 128), dtype=np.float32),
                            **shard[hg]))

    res = run_bass_kernel_spmd(nc, in_maps, core_ids=list(range(NCORES)),
                               **_CACHE.get("run_kwargs", {}))
    _CACHE["last_result"] = res
    out = np.empty((B, T, C), dtype=np.float32)
    for b in range(B):
        out[b] = (res.results[2 * b]["o"] + res.results[2 * b + 1]["o"]
                  + bias_final)
    return out


# revision 30
# speedup vs baseline: 1.1634x; 1.0102x over previous
"""Trainium2 Bass kernel for a causal attention head block (B=4, T=2048, C=2048,
H=16, D=128) with RoPE (single fixed position, folded into weights on host).

Sharding: 8 cores = 4 batches x 2 head-groups (8 heads each).
Per core: QKV projection (f32r matmuls), causal attention with exp-softmax
(no max subtraction -- scores are small), out-projection partial. Host sums
the two per-batch partials and adds the folded bias.

All matmuls run in float32r (TRN2 reduced-precision fp32 mode): ~bf16 speed at
moving-dim >= 256, ~1.5e-4 relative accuracy per matmul.
"""
import numpy as np

B, T, C, H, D = 4, 2048, 2048, 16, 128
ROPE_BASE = 10000.0
HG = H // 2            # heads per core: 8
JQ = HG * D            # 1024 q (or k, or v) channels per core
NCORES = 8
NCT = C // 128         # 16 contraction tiles
NTT = T // 128         # 16 token tiles
NTC = T // 512         # 4 token chunks of 512

_CACHE = {}


def _build_nc():
    import concourse.bass as bass
    import concourse.mybir as mybir
    import concourse.tile as tile
    from concourse import bacc

    f32, f32r = mybir.dt.float32, mybir.dt.float32r
    ds, ts = bass.ds, bass.ts
    Exp = mybir.ActivationFunctionType.Exp
    Ident = mybir.ActivationFunctionType.Identity
    mult = mybir.AluOpType.mult
    add = mybir.AluOpType.add

    nc = bacc.Bacc("TRN2", target_bir_lowering=False, debug=False)
    xT = nc.dram_tensor("xT", [C, T], f32r, kind="ExternalInput").ap()
    WqkT = nc.dram_tensor("WqkT", [C, 2 * JQ], f32r, kind="ExternalInput").ap()
    WvT = nc.dram_tensor("WvT", [C, JQ], f32r, kind="ExternalInput").ap()
    WoT = nc.dram_tensor("WoT", [JQ, C], f32r, kind="ExternalInput").ap()
    bq = nc.dram_tensor("bq", [JQ, 1], f32, kind="ExternalInput").ap()
    msk = nc.dram_tensor("msk", [128, 4 * 512], f32r, kind="ExternalInput").ap()
    ones_in = nc.dram_tensor("ones_in", [128, 128], f32r, kind="ExternalInput").ap()
    qk_sp = nc.dram_tensor("qk_sp", [2 * JQ, T], f32r).ap()   # [q;k]^T spill
    v_sp = nc.dram_tensor("v_sp", [T, JQ], f32r).ap()         # V spill [t, jv]
    o = nc.dram_tensor("o", [T, C], f32, kind="ExternalOutput").ap()

    with tile.TileContext(nc) as tc:
        with tc.tile_pool(name="const", bufs=1) as cpool:
            mask_t = cpool.tile([128, 4 * 512], f32r, tag="mask")
            nc.sync.dma_start(mask_t[:], msk[:])
            ones2d = cpool.tile([128, 128], f32r, tag="ones2d")
            nc.sync.dma_start(ones2d[:], ones_in[:])
            ones_col = ones2d[:, 0:1]
            ones_row = ones2d[0:1, :]
            bq_t = []
            for j in range(JQ // 128):
                t_ = cpool.tile([128, 1], f32, tag=f"bq{j}")
                nc.sync.dma_start(t_[:], bq[ts(j, 128), :])
                bq_t.append(t_)

            # PE warmup: dummy fp32 matmuls with no input deps keep the
            # HAM activity monitor busy during the initial DMA wait so
            # phase A's first real matmuls run at full clock.
            wtile = cpool.tile([128, 640], f32, tag="warm")
            nc.vector.memset(wtile[:], 0.0)
            with tc.tile_pool(name="pwarm", bufs=1, space="PSUM") as pwarm:
                ps_w = pwarm.tile([128, 512], f32, tag="pw", name="pw")
                for wi in range(8):
                    nc.tensor.matmul(ps_w[:], wtile[:, 0:128],
                                     wtile[:, 128:640], start=(wi == 0),
                                     stop=(wi == 7))

            # ---------------- Phase A: projections ----------------
            # Head-0 attention inputs live below the xt stack slot so their
            # DMAs can prefetch during phase A (disjoint SBUF space).
            kv0cm = tc.tile_pool(name="kv0", bufs=1)
            kv0pool = kv0cm.__enter__()
            kt0 = kv0pool.tile([128, T], f32r, tag="kt0", name="kt0")
            vh0 = kv0pool.tile([128, T], f32r, tag="vh0", name="vh0")
            qc0 = kv0pool.tile([128, 512], f32r, tag="qc0", name="qc0")
            with tc.tile_pool(name="xt", bufs=1) as xpool:
                # A-V first: V[t, jv] = x^T[c, t].T @ Wv^T[c, jv].
                # Interleave xT and first-chunk WvT DMAs so PE starts early.
                xt = [None] * NCT
                with tc.tile_pool(name="wv", bufs=19) as wvpool, \
                     tc.tile_pool(name="vst", bufs=4) as vspool, \
                     tc.tile_pool(name="psV", bufs=5, space="PSUM") as psvpool:
                    for vch in range(JQ // 512):     # 2 chunks of 512
                        wvs = []
                        for ci in range(NCT):
                            if vch == 0:
                                t_ = xpool.tile([128, T], f32r, tag=f"x{ci}",
                                                name=f"x{ci}")
                                nc.sync.dma_start(t_[:], xT[ts(ci, 128), :])
                                xt[ci] = t_
                            w_ = wvpool.tile([128, 512], f32r, tag="wv",
                                             name=f"wv{vch}_{ci}")
                            nc.sync.dma_start(
                                w_[:], WvT[ts(ci, 128), ds(vch * 512, 512)])
                            wvs.append(w_)
                        for tt in range(NTT):
                            ps = psvpool.tile([128, 512], f32, tag="psv")
                            for ci in range(NCT):
                                nc.tensor.matmul(
                                    ps[:], xt[ci][:, ts(tt, 128)], wvs[ci][:],
                                    start=(ci == 0), stop=(ci == NCT - 1))
                            st = vspool.tile([128, 512], f32r, tag="vst")
                            nc.vector.tensor_copy(st[:], ps[:])
                            nc.gpsimd.dma_start(
                                v_sp[ts(tt, 128), ds(vch * 512, 512)], st[:])
                # A-QK: qk^T[j, t] = Wqk^T[c, j].T @ x^T[c, t]  (+ bias on q)
                # Group order q0, k0, q1, k1 so attention heads 0-3 unblock
                # after two groups.
                with tc.tile_pool(name="wqk", bufs=18) as wpool, \
                     tc.tile_pool(name="qkst", bufs=6) as spool, \
                     tc.tile_pool(name="psA", bufs=5, space="PSUM") as pspool:
                    for jg_i, jg in enumerate((0, 2, 1, 3)):
                        wts = []
                        for ci in range(NCT):
                            w_ = wpool.tile([128, 512], f32r, tag="w",
                                            name=f"w{jg}_{ci}")
                            nc.sync.dma_start(
                                w_[:], WqkT[ts(ci, 128), ds(jg * 512, 512)])
                            wts.append(w_)
                        for jj in range(4):
                            jt = jg * 4 + jj
                            pss_l = [pspool.tile([128, 512], f32, tag="ps",
                                                 name=f"ps{jt}_{t2}")
                                     for t2 in range(NTC)]
                            for ci in range(NCT):
                                for tch in range(NTC):
                                    nc.tensor.matmul(
                                        pss_l[tch][:], wts[ci][:, ts(jj, 128)],
                                        xt[ci][:, ts(tch, 512)],
                                        start=(ci == 0), stop=(ci == NCT - 1))
                            for tch in range(NTC):
                                st = spool.tile([128, 512], f32r, tag="st")
                                if jt < JQ // 128:   # q tile: bias add
                                    nc.scalar.activation(
                                        st[:], pss_l[tch][:], Ident,
                                        bias=bq_t[jt][:, 0:1])
                                else:                # k tile: plain copy
                                    nc.scalar.copy(st[:], pss_l[tch][:])
                                nc.gpsimd.dma_start(
                                    qk_sp[ts(jt, 128), ts(tch, 512)], st[:])
                        if jg_i == 1:
                            # prefetch head-0 attention inputs mid-phase-A
                            nc.sync.dma_start(
                                kt0[:], qk_sp[ds(JQ, 128), :])
                            nc.sync.dma_start(
                                vh0[:].rearrange("p (n d) -> p n d", d=128),
                                v_sp[:, ds(0, 128)].rearrange(
                                    "(n p) d -> p n d", p=128))
                            nc.sync.dma_start(qc0[:], qk_sp[ds(0, 128),
                                                            ds(0, 512)])

            # ---------------- Phase B: attention ----------------
            with tc.tile_pool(name="ysb", bufs=1) as ypool:
                y_t = [ypool.tile([128, T], f32r, tag=f"y{h}", name=f"y{h}")
                       for h in range(HG)]
                with tc.tile_pool(name="kv", bufs=2) as kvpool, \
                     tc.tile_pool(name="qc", bufs=2) as qcpool, \
                     tc.tile_pool(name="es", bufs=6) as espool, \
                     tc.tile_pool(name="nrm", bufs=3) as npool, \
                     tc.tile_pool(name="psS", bufs=3, space="PSUM") as pss, \
                     tc.tile_pool(name="psY", bufs=2, space="PSUM") as psy, \
                     tc.tile_pool(name="psD", bufs=2, space="PSUM") as psd, \
                     tc.tile_pool(name="psB", bufs=1, space="PSUM") as psb:

                    def emit_norm(pend):
                        # Deferred per-chunk softmax normalization: runs the
                        # reciprocal off the PE critical path (the broadcast
                        # matmul is emitted a few matmuls into the NEXT chunk,
                        # by which time the DVE reciprocal has completed).
                        ps_y_, ps_d_, h_, ci_ = pend
                        rec = npool.tile([1, 512], f32, tag="rec", name="rec")
                        nc.vector.reciprocal_approx_fast(rec[:], ps_d_[:])
                        recr = npool.tile([1, 512], f32r, tag="recr",
                                          name="recr")
                        nc.vector.tensor_copy(recr[:], rec[:])
                        ps_b = psb.tile([128, 512], f32, tag="pb", name="pb")
                        nc.tensor.matmul(ps_b[:], ones_row, recr[:],
                                         start=True, stop=True)
                        bc = npool.tile([128, 512], f32r, tag="bc", name="bc")
                        nc.scalar.copy(bc[:], ps_b[:])
                        nc.vector.tensor_tensor(
                            y_t[h_][:, ds(ci_ * 512, 512)], ps_y_[:], bc[:],
                            mult)

                    pending = None
                    carry = None
                    for h in range(HG):
                        if h == 0:
                            kt = kt0
                        else:
                            kt = kvpool.tile([128, T], f32r, tag="kt")
                            nc.sync.dma_start(
                                kt[:], qk_sp[ds(JQ + h * 128, 128), :])
                        if h == 0:
                            vh = vh0
                        else:
                            vh = kvpool.tile([128, T], f32r, tag="vh")
                            nc.sync.dma_start(
                                vh[:].rearrange("p (n d) -> p n d", d=128),
                                v_sp[:, ds(h * 128, 128)].rearrange(
                                    "(n p) d -> p n d", p=128))
                        for ci in range(NTC):
                            if h == 0 and ci == 0:
                                qc = qc0
                            else:
                                qc = qcpool.tile([128, 512], f32r, tag="qc")
                                nc.sync.dma_start(
                                    qc[:],
                                    qk_sp[ds(h * 128, 128), ds(ci * 512, 512)])
                            ps_y = psy.tile([128, 512], f32, tag="py")
                            ps_d = psd.tile([1, 512], f32, tag="pd")
                            njt = 4 * (ci + 1)
                            norm_at = min(4, njt - 1)
                            for jt in range(njt):
                                jx = jt
                                ps_s = pss.tile([128, 512], f32, tag="pss")
                                nc.tensor.matmul(ps_s[:], kt[:, ts(jt, 128)],
                                                 qc[:], start=True, stop=True)
                                if jx == norm_at and pending is not None:
                                    emit_norm(pending)
                                    pending = None
                                if carry is not None:
                                    (c_es, c_py, c_pd, c_vs, c_st, c_first,
                                     c_h, c_ci) = carry
                                    nc.tensor.matmul(
                                        c_pd[:], ones_col, c_es[:],
                                        start=c_first, stop=c_st)
                                    nc.tensor.matmul(
                                        c_py[:], c_vs, c_es[:],
                                        start=c_first, stop=c_st)
                                    if c_st:
                                        pending = (c_py, c_pd, c_h, c_ci)
                                es = espool.tile([128, 512], f32r, tag="es")
                                nc.scalar.activation(es[:], ps_s[:], Exp)
                                off = jt * 128 - ci * 512
                                if off >= 0:   # diagonal band: apply mask
                                    oi = off // 128
                                    nc.vector.tensor_tensor(
                                        es[:], es[:],
                                        mask_t[:, ds(oi * 512, 512)], mult)
                                carry = (es, ps_y, ps_d, vh[:, ts(jt, 128)],
                                         jx == njt - 1, jx == 0, h, ci)
                    # drain the last carry + norm
                    c_es, c_py, c_pd, c_vs, c_st, c_first, c_h, c_ci = carry
                    nc.tensor.matmul(c_pd[:], ones_col, c_es[:],
                                     start=c_first, stop=True)
                    nc.tensor.matmul(c_py[:], c_vs, c_es[:],
                                     start=c_first, stop=True)
                    if pending is not None:
                        emit_norm(pending)
                    emit_norm((c_py, c_pd, c_h, c_ci))

                # ---------------- Phase C: out-projection ----------------
                with tc.tile_pool(name="woc", bufs=1) as wopool, \
                     tc.tile_pool(name="ost", bufs=6) as ospool, \
                     tc.tile_pool(name="psO", bufs=5, space="PSUM") as pso:
                    wo_t = []
                    for ch in range(HG):
                        w_ = wopool.tile([128, C], f32r, tag=f"wo{ch}",
                                         name=f"wo{ch}")
                        nc.sync.dma_start(w_[:], WoT[ts(ch, 128), :])
                        wo_t.append(w_)
                    for tt in range(NTT):
                        po_l = [pso.tile([128, 512], f32, tag="po",
                                         name=f"po{tt}_{c2}")
                                for c2 in range(C // 512)]
                        for ch in range(HG):
                            for cch in range(C // 512):
                                nc.tensor.matmul(
                                    po_l[cch][:], y_t[ch][:, ts(tt, 128)],
                                    wo_t[ch][:, ds(cch * 512, 512)],
                                    start=(ch == 0), stop=(ch == HG - 1))
                        for cch in range(C // 512):
                            st = ospool.tile([128, 512], f32, tag="ost")
                            nc.vector.tensor_copy(st[:], po_l[cch][:])
                            nc.gpsimd.dma_start(
                                o[ts(tt, 128), ds(cch * 512, 512)], st[:])
            kv0cm.__exit__(None, None, None)
    nc.compile()
    return nc


def _rope_matrix():
    inv_freq = 1.0 / (ROPE_BASE ** (np.arange(0, D, 2, dtype=np.float64) / D))
    freqs = np.float64(T) * inv_freq
    emb = np.concatenate([freqs, freqs])
    cos, sin = np.cos(emb), np.sin(emb)
    R = np.zeros((D, D))
    for j in range(D):
        R[j, j] = cos[j]
    for j in range(64):
        R[j, 2 * j + 1] += -sin[j]
    for j in range(64, 128):
        R[j, 2 * (j - 64)] += sin[j]
    return R


def _host_mask():
    m = np.zeros((128, 4 * 512), dtype=np.float32)
    ii = np.arange(512)
    jj = np.arange(128)
    for oi, off in enumerate((0, 128, 256, 384)):
        m[:, oi * 512:(oi + 1) * 512] = (jj[:, None] + off <= ii[None, :])
    return m


def kernel(x, Wqkv, bqkv, Wout, bout):
    from concourse.bass_utils import run_bass_kernel_spmd

    if "nc" not in _CACHE:
        _CACHE["nc"] = _build_nc()
    nc = _CACHE["nc"]

    x = np.asarray(x, dtype=np.float32)
    Wqkv64 = np.asarray(Wqkv, dtype=np.float64)
    bqkv64 = np.asarray(bqkv, dtype=np.float64)
    Wout64 = np.asarray(Wout, dtype=np.float64)
    bout64 = np.asarray(bout, dtype=np.float64)

    R = _rope_matrix()
    scale = 1.0 / np.sqrt(np.float64(D))
    Wq = Wqkv64[:C].reshape(H, D, C)
    Wk = Wqkv64[C:2 * C].reshape(H, D, C)
    Wv = Wqkv64[2 * C:].reshape(H, D, C)
    bqv = bqkv64[:C].reshape(H, D)
    bv = bqkv64[2 * C:]

    Wq_f = np.einsum('jk,hkc->hjc', R, Wq) * scale
    bq_f = np.einsum('jk,hk->hj', R, bqv) * scale
    Wk_f = np.einsum('jk,hkc->hjc', R, Wk)
    bias_final = (bout64 + Wout64 @ bv).astype(np.float32)

    mask = _host_mask()
    in_maps = []
    xTb = [np.ascontiguousarray(x[b].T) for b in range(B)]
    shard = {}
    for hg in range(2):
        hs = slice(hg * HG, (hg + 1) * HG)
        wqk = np.concatenate(
            [Wq_f[hs].reshape(JQ, C), Wk_f[hs].reshape(JQ, C)], axis=0)
        shard[hg] = dict(
            WqkT=np.ascontiguousarray(wqk.T.astype(np.float32)),
            WvT=np.ascontiguousarray(
                Wv[hs].reshape(JQ, C).T.astype(np.float32)),
            WoT=np.ascontiguousarray(
                Wout64[:, hg * JQ:(hg + 1) * JQ].T.astype(np.float32)),
            bq=bq_f[hs].reshape(JQ, 1).astype(np.float32),
        )
    for core in range(NCORES):
        b, hg = core // 2, core % 2
        in_maps.append(dict(xT=xTb[b], msk=mask,
                            ones_in=np.ones((128, 128), dtype=np.float32),
                            **shard[hg]))

    res = run_bass_kernel_spmd(nc, in_maps, core_ids=list(range(NCORES)),
                               **_CACHE.get("run_kwargs", {}))
    _CACHE["last_result"] = res
    out = np.empty((B, T, C), dtype=np.float32)
    for b in range(B):
        out[b] = (res.results[2 * b]["o"] + res.results[2 * b + 1]["o"]
                  + bias_final)
    return out


# revision 31
# speedup vs baseline: 1.1934x; 1.0258x over previous
"""Trainium2 Bass kernel for a causal attention head block (B=4, T=2048, C=2048,
H=16, D=128) with RoPE (single fixed position, folded into weights on host).

Sharding: 8 cores = 4 batches x 2 head-groups (8 heads each).
Per core: QKV projection (f32r matmuls), causal attention with exp-softmax
(no max subtraction -- scores are small), out-projection partial. Host sums
the two per-batch partials and adds the folded bias.

All matmuls run in float32r (TRN2 reduced-precision fp32 mode): ~bf16 speed at
moving-dim >= 256, ~1.5e-4 relative accuracy per matmul.
"""
import numpy as np

B, T, C, H, D = 4, 2048, 2048, 16, 128
ROPE_BASE = 10000.0
HG = H // 2            # heads per core: 8
JQ = HG * D            # 1024 q (or k, or v) channels per core
NCORES = 8
NCT = C // 128         # 16 contraction tiles
NTT = T // 128         # 16 token tiles
NTC = T // 512         # 4 token chunks of 512

_CACHE = {}


def _build_nc():
    import concourse.bass as bass
    import concourse.mybir as mybir
    import concourse.tile as tile
    from concourse import bacc

    f32, f32r = mybir.dt.float32, mybir.dt.float32r
    ds, ts = bass.ds, bass.ts
    Exp = mybir.ActivationFunctionType.Exp
    Ident = mybir.ActivationFunctionType.Identity
    mult = mybir.AluOpType.mult
    add = mybir.AluOpType.add

    nc = bacc.Bacc("TRN2", target_bir_lowering=False, debug=False)
    xT = nc.dram_tensor("xT", [C, T], f32r, kind="ExternalInput").ap()
    WqkT = nc.dram_tensor("WqkT", [C, 2 * JQ], f32r, kind="ExternalInput").ap()
    WvT = nc.dram_tensor("WvT", [C, JQ], f32r, kind="ExternalInput").ap()
    WoT = nc.dram_tensor("WoT", [JQ, C], f32r, kind="ExternalInput").ap()
    bq = nc.dram_tensor("bq", [JQ, 1], f32, kind="ExternalInput").ap()
    msk = nc.dram_tensor("msk", [128, 4 * 512], f32r, kind="ExternalInput").ap()
    ones_in = nc.dram_tensor("ones_in", [128, 128], f32r, kind="ExternalInput").ap()
    qk_sp = nc.dram_tensor("qk_sp", [2 * JQ, T], f32r).ap()   # [q;k]^T spill
    v_sp = nc.dram_tensor("v_sp", [T, JQ], f32r).ap()         # V spill [t, jv]
    o = nc.dram_tensor("o", [T, C], f32, kind="ExternalOutput").ap()

    with tile.TileContext(nc) as tc:
        with tc.tile_pool(name="const", bufs=1) as cpool:
            mask_t = cpool.tile([128, 4 * 512], f32r, tag="mask")
            nc.sync.dma_start(mask_t[:], msk[:])
            ones2d = cpool.tile([128, 128], f32r, tag="ones2d")
            nc.sync.dma_start(ones2d[:], ones_in[:])
            ones_col = ones2d[:, 0:1]
            ones_row = ones2d[0:1, :]
            bq_t = []
            for j in range(JQ // 128):
                t_ = cpool.tile([128, 1], f32, tag=f"bq{j}")
                nc.sync.dma_start(t_[:], bq[ts(j, 128), :])
                bq_t.append(t_)

            # PE warmup: dummy fp32 matmuls with no input deps keep the
            # HAM activity monitor busy during the initial DMA wait so
            # phase A's first real matmuls run at full clock.
            wtile = cpool.tile([128, 640], f32, tag="warm")
            nc.vector.memset(wtile[:], 0.0)
            with tc.tile_pool(name="pwarm", bufs=1, space="PSUM") as pwarm:
                ps_w = pwarm.tile([128, 512], f32, tag="pw", name="pw")
                for wi in range(8):
                    nc.tensor.matmul(ps_w[:], wtile[:, 0:128],
                                     wtile[:, 128:640], start=(wi == 0),
                                     stop=(wi == 7))

            # ---------------- Phase A: projections ----------------
            # Head-0 attention inputs live below the xt stack slot so their
            # DMAs can prefetch during phase A (disjoint SBUF space).
            kv0cm = tc.tile_pool(name="kv0", bufs=1)
            kv0pool = kv0cm.__enter__()
            kt0 = kv0pool.tile([128, T], f32r, tag="kt0", name="kt0")
            vh0 = kv0pool.tile([128, T], f32r, tag="vh0", name="vh0")
            qc0 = kv0pool.tile([128, 512], f32r, tag="qc0", name="qc0")
            with tc.tile_pool(name="xt", bufs=1) as xpool:
                # A-V first: V[t, jv] = x^T[c, t].T @ Wv^T[c, jv].
                # Interleave xT and first-chunk WvT DMAs so PE starts early.
                xt = [None] * NCT
                with tc.tile_pool(name="wv", bufs=19) as wvpool, \
                     tc.tile_pool(name="vst", bufs=4) as vspool, \
                     tc.tile_pool(name="psV", bufs=5, space="PSUM") as psvpool:
                    for vch in range(JQ // 512):     # 2 chunks of 512
                        wvs = []
                        for ci in range(NCT):
                            if vch == 0:
                                t_ = xpool.tile([128, T], f32r, tag=f"x{ci}",
                                                name=f"x{ci}")
                                # first token-column block right away so the
                                # first matmuls unblock after ~1 MB of DMA
                                nc.sync.dma_start(t_[:, 0:512],
                                                 xT[ts(ci, 128), 0:512])
                                xt[ci] = t_
                            w_ = wvpool.tile([128, 512], f32r, tag="wv",
                                             name=f"wv{vch}_{ci}")
                            nc.sync.dma_start(
                                w_[:], WvT[ts(ci, 128), ds(vch * 512, 512)])
                            wvs.append(w_)
                        if vch == 0:
                            for tcol in range(1, NTC):
                                for ci in range(NCT):
                                    nc.sync.dma_start(
                                        xt[ci][:, ts(tcol, 512)],
                                        xT[ts(ci, 128), ts(tcol, 512)])
                        for tt in range(NTT):
                            ps = psvpool.tile([128, 512], f32, tag="psv")
                            for ci in range(NCT):
                                nc.tensor.matmul(
                                    ps[:], xt[ci][:, ts(tt, 128)], wvs[ci][:],
                                    start=(ci == 0), stop=(ci == NCT - 1))
                            st = vspool.tile([128, 512], f32r, tag="vst")
                            nc.vector.tensor_copy(st[:], ps[:])
                            nc.gpsimd.dma_start(
                                v_sp[ts(tt, 128), ds(vch * 512, 512)], st[:])
                # A-QK: qk^T[j, t] = Wqk^T[c, j].T @ x^T[c, t]  (+ bias on q)
                # Group order q0, k0, q1, k1 so attention heads 0-3 unblock
                # after two groups.
                with tc.tile_pool(name="wqk", bufs=18) as wpool, \
                     tc.tile_pool(name="qkst", bufs=6) as spool, \
                     tc.tile_pool(name="psA", bufs=5, space="PSUM") as pspool:
                    for jg_i, jg in enumerate((0, 2, 1, 3)):
                        wts = []
                        for ci in range(NCT):
                            w_ = wpool.tile([128, 512], f32r, tag="w",
                                            name=f"w{jg}_{ci}")
                            nc.sync.dma_start(
                                w_[:], WqkT[ts(ci, 128), ds(jg * 512, 512)])
                            wts.append(w_)
                        for jj in range(4):
                            jt = jg * 4 + jj
                            pss_l = [pspool.tile([128, 512], f32, tag="ps",
                                                 name=f"ps{jt}_{t2}")
                                     for t2 in range(NTC)]
                            for ci in range(NCT):
                                for tch in range(NTC):
                                    nc.tensor.matmul(
                                        pss_l[tch][:], wts[ci][:, ts(jj, 128)],
                                        xt[ci][:, ts(tch, 512)],
                                        start=(ci == 0), stop=(ci == NCT - 1))
                            for tch in range(NTC):
                                st = spool.tile([128, 512], f32r, tag="st")
                                if jt < JQ // 128:   # q tile: bias add
                                    nc.scalar.activation(
                                        st[:], pss_l[tch][:], Ident,
                                        bias=bq_t[jt][:, 0:1])
                                else:                # k tile: plain copy
                                    nc.scalar.copy(st[:], pss_l[tch][:])
                                nc.gpsimd.dma_start(
                                    qk_sp[ts(jt, 128), ts(tch, 512)], st[:])
                        if jg_i == 1:
                            # prefetch head-0 attention inputs mid-phase-A
                            nc.sync.dma_start(
                                kt0[:], qk_sp[ds(JQ, 128), :])
                            nc.sync.dma_start(
                                vh0[:].rearrange("p (n d) -> p n d", d=128),
                                v_sp[:, ds(0, 128)].rearrange(
                                    "(n p) d -> p n d", p=128))
                            nc.sync.dma_start(qc0[:], qk_sp[ds(0, 128),
                                                            ds(0, 512)])

            # ---------------- Phase B: attention ----------------
            with tc.tile_pool(name="ysb", bufs=1) as ypool:
                y_t = [ypool.tile([128, T], f32r, tag=f"y{h}", name=f"y{h}")
                       for h in range(HG)]
                with tc.tile_pool(name="kv", bufs=2) as kvpool, \
                     tc.tile_pool(name="qc", bufs=2) as qcpool, \
                     tc.tile_pool(name="es", bufs=6) as espool, \
                     tc.tile_pool(name="nrm", bufs=3) as npool, \
                     tc.tile_pool(name="psS", bufs=3, space="PSUM") as pss, \
                     tc.tile_pool(name="psY", bufs=2, space="PSUM") as psy, \
                     tc.tile_pool(name="psD", bufs=2, space="PSUM") as psd, \
                     tc.tile_pool(name="psB", bufs=1, space="PSUM") as psb:

                    def emit_norm(pend):
                        # Deferred per-chunk softmax normalization: runs the
                        # reciprocal off the PE critical path (the broadcast
                        # matmul is emitted a few matmuls into the NEXT chunk,
                        # by which time the DVE reciprocal has completed).
                        ps_y_, ps_d_, h_, ci_ = pend
                        rec = npool.tile([1, 512], f32, tag="rec", name="rec")
                        nc.vector.reciprocal_approx_fast(rec[:], ps_d_[:])
                        recr = npool.tile([1, 512], f32r, tag="recr",
                                          name="recr")
                        nc.vector.tensor_copy(recr[:], rec[:])
                        ps_b = psb.tile([128, 512], f32, tag="pb", name="pb")
                        nc.tensor.matmul(ps_b[:], ones_row, recr[:],
                                         start=True, stop=True)
                        bc = npool.tile([128, 512], f32r, tag="bc", name="bc")
                        nc.scalar.copy(bc[:], ps_b[:])
                        nc.vector.tensor_tensor(
                            y_t[h_][:, ds(ci_ * 512, 512)], ps_y_[:], bc[:],
                            mult)

                    pending = None
                    carry = None
                    for h in range(HG):
                        if h == 0:
                            kt = kt0
                        else:
                            kt = kvpool.tile([128, T], f32r, tag="kt")
                            nc.sync.dma_start(
                                kt[:], qk_sp[ds(JQ + h * 128, 128), :])
                        if h == 0:
                            vh = vh0
                        else:
                            vh = kvpool.tile([128, T], f32r, tag="vh")
                            nc.sync.dma_start(
                                vh[:].rearrange("p (n d) -> p n d", d=128),
                                v_sp[:, ds(h * 128, 128)].rearrange(
                                    "(n p) d -> p n d", p=128))
                        for ci in range(NTC):
                            if h == 0 and ci == 0:
                                qc = qc0
                            else:
                                qc = qcpool.tile([128, 512], f32r, tag="qc")
                                nc.sync.dma_start(
                                    qc[:],
                                    qk_sp[ds(h * 128, 128), ds(ci * 512, 512)])
                            ps_y = psy.tile([128, 512], f32, tag="py")
                            ps_d = psd.tile([1, 512], f32, tag="pd")
                            njt = 4 * (ci + 1)
                            norm_at = min(4, njt - 1)
                            for jt in range(njt):
                                jx = jt
                                ps_s = pss.tile([128, 512], f32, tag="pss")
                                nc.tensor.matmul(ps_s[:], kt[:, ts(jt, 128)],
                                                 qc[:], start=True, stop=True)
                                if jx == norm_at and pending is not None:
                                    emit_norm(pending)
                                    pending = None
                                if carry is not None:
                                    (c_es, c_py, c_pd, c_vs, c_st, c_first,
                                     c_h, c_ci) = carry
                                    nc.tensor.matmul(
                                        c_pd[:], ones_col, c_es[:],
                                        start=c_first, stop=c_st)
                                    nc.tensor.matmul(
                                        c_py[:], c_vs, c_es[:],
                                        start=c_first, stop=c_st)
                                    if c_st:
                                        pending = (c_py, c_pd, c_h, c_ci)
                                es = espool.tile([128, 512], f32r, tag="es")
                                nc.scalar.activation(es[:], ps_s[:], Exp)
                                off = jt * 128 - ci * 512
                                if off >= 0:   # diagonal band: apply mask
                                    oi = off // 128
                                    nc.vector.tensor_tensor(
                                        es[:], es[:],
                                        mask_t[:, ds(oi * 512, 512)], mult)
                                carry = (es, ps_y, ps_d, vh[:, ts(jt, 128)],
                                         jx == njt - 1, jx == 0, h, ci)
                    # drain the last carry + norm
                    c_es, c_py, c_pd, c_vs, c_st, c_first, c_h, c_ci = carry
                    nc.tensor.matmul(c_pd[:], ones_col, c_es[:],
                                     start=c_first, stop=True)
                    nc.tensor.matmul(c_py[:], c_vs, c_es[:],
                                     start=c_first, stop=True)
                    if pending is not None:
                        emit_norm(pending)
                    emit_norm((c_py, c_pd, c_h, c_ci))

                # ---------------- Phase C: out-projection ----------------
                with tc.tile_pool(name="woc", bufs=1) as wopool, \
                     tc.tile_pool(name="ost", bufs=6) as ospool, \
                     tc.tile_pool(name="psO", bufs=5, space="PSUM") as pso:
                    wo_t = []
                    for ch in range(HG):
                        w_ = wopool.tile([128, C], f32r, tag=f"wo{ch}",
                                         name=f"wo{ch}")
                        nc.sync.dma_start(w_[:], WoT[ts(ch, 128), :])
                        wo_t.append(w_)
                    for tt in range(NTT):
                        po_l = [pso.tile([128, 512], f32, tag="po",
                                         name=f"po{tt}_{c2}")
                                for c2 in range(C // 512)]
                        for ch in range(HG):
                            for cch in range(C // 512):
                                nc.tensor.matmul(
                                    po_l[cch][:], y_t[ch][:, ts(tt, 128)],
                                    wo_t[ch][:, ds(cch * 512, 512)],
                                    start=(ch == 0), stop=(ch == HG - 1))
                        for cch in range(C // 512):
                            st = ospool.tile([128, 512], f32, tag="ost")
                            nc.vector.tensor_copy(st[:], po_l[cch][:])
                            nc.gpsimd.dma_start(
                                o[ts(tt, 128), ds(cch * 512, 512)], st[:])
            kv0cm.__exit__(None, None, None)
    nc.compile()
    return nc


def _rope_matrix():
    inv_freq = 1.0 / (ROPE_BASE ** (np.arange(0, D, 2, dtype=np.float64) / D))
    freqs = np.float64(T) * inv_freq
    emb = np.concatenate([freqs, freqs])
    cos, sin = np.cos(emb), np.sin(emb)
    R = np.zeros((D, D))
    for j in range(D):
        R[j, j] = cos[j]
    for j in range(64):
        R[j, 2 * j + 1] += -sin[j]
    for j in range(64, 128):
        R[j, 2 * (j - 64)] += sin[j]
    return R


def _host_mask():
    m = np.zeros((128, 4 * 512), dtype=np.float32)
    ii = np.arange(512)
    jj = np.arange(128)
    for oi, off in enumerate((0, 128, 256, 384)):
        m[:, oi * 512:(oi + 1) * 512] = (jj[:, None] + off <= ii[None, :])
    return m


def kernel(x, Wqkv, bqkv, Wout, bout):
    from concourse.bass_utils import run_bass_kernel_spmd

    if "nc" not in _CACHE:
        _CACHE["nc"] = _build_nc()
    nc = _CACHE["nc"]

    x = np.asarray(x, dtype=np.float32)
    Wqkv64 = np.asarray(Wqkv, dtype=np.float64)
    bqkv64 = np.asarray(bqkv, dtype=np.float64)
    Wout64 = np.asarray(Wout, dtype=np.float64)
    bout64 = np.asarray(bout, dtype=np.float64)

    R = _rope_matrix()
    scale = 1.0 / np.sqrt(np.float64(D))
    Wq = Wqkv64[:C].reshape(H, D, C)
    Wk = Wqkv64[C:2 * C].reshape(H, D, C)
    Wv = Wqkv64[2 * C:].reshape(H, D, C)
    bqv = bqkv64[:C].reshape(H, D)
    bv = bqkv64[2 * C:]

    Wq_f = np.einsum('jk,hkc->hjc', R, Wq) * scale
    bq_f = np.einsum('jk,hk->hj', R, bqv) * scale
    Wk_f = np.einsum('jk,hkc->hjc', R, Wk)
    bias_final = (bout64 + Wout64 @ bv).astype(np.float32)

    mask = _host_mask()
    in_maps = []
    xTb = [np.ascontiguousarray(x[b].T) for b in range(B)]
    shard = {}
    for hg in range(2):
        hs = slice(hg * HG, (hg + 1) * HG)
        wqk = np.concatenate(
            [Wq_f[hs].reshape(JQ, C), Wk_f[hs].reshape(JQ, C)], axis=0)
        shard[hg] = dict(
            WqkT=np.ascontiguousarray(wqk.T.astype(np.float32)),
            WvT=np.ascontiguousarray(
                Wv[hs].reshape(JQ, C).T.astype(np.float32)),
            WoT=np.ascontiguousarray(
                Wout64[:, hg * JQ:(hg + 1) * JQ].T.astype(np.float32)),
            bq=bq_f[hs].reshape(JQ, 1).astype(np.float32),
        )
    for core in range(NCORES):
        b, hg = core // 2, core % 2
        in_maps.append(dict(xT=xTb[b], msk=mask,
                            ones_in=np.ones((128, 128), dtype=np.float32),
                            **shard[hg]))

    res = run_bass_kernel_spmd(nc, in_maps, core_ids=list(range(NCORES)),
                               **_CACHE.get("run_kwargs", {}))
    _CACHE["last_result"] = res
    out = np.empty((B, T, C), dtype=np.float32)
    for b in range(B):
        out[b] = (res.results[2 * b]["o"] + res.results[2 * b + 1]["o"]
                  + bias_final)
    return out
